# revision 14
# baseline (speedup 1.0000x reference)
"""Trainium2 Bass kernel for BaseTextureNCA (neural cellular automaton step).

Math:
  y  = depthwise 3x3 conv of x with 4 fixed filters (circular pad)   [b,48,H,W]
  h  = relu(W1 @ y + b1)                                             [b,96,H,W]
  dy = W2 @ h                                                        [b,12,H,W]
  out = x + dy * floor(rand_u + 0.5)

Kernel formulation (per core = one batch image), v2:
  - All matmul operands are bf16 (1 cycle/row on the PE vs ~2.4 for f32r);
    PSUM accumulates fp32.
  - Fold the fixed filters into W1 -> conv1 is ONE matmul per output row:
    contraction K=109 = 12 channels x 9 taps + 1 mask row.  The 9 taps are
    materialized as 9 shifted window copies in the xb tile (partition
    p = (kx*3+ky)*12+c), loaded by a single strided DMA from a circularly
    padded bf16 image xpad2 [C, H+2, W+2] staged in DRAM.
  - The stochastic mask is folded into conv1 as contraction row 108 with
    t = -1e6 where rand_u < 0.5 else 0: relu(pre + t) == relu(pre)*mask.
  - Everything runs in (128,32) column-tiled PE mode (tile_position): conv1
    is 3 concurrent 32-hidden-wide matmuls; conv2 packs 4 consecutive rows
    into ONE psum bank via 4 column tiles at partition offsets 0/32/64/96
    (weights zero-padded 12->32 so the bank is fully written).  Column tiles
    execute concurrently, and per-tile LDWEIGHTS (32 cols, bf16) is cheap
    and overlaps other tiles' matmuls.
  - conv2 appends a 12x12 identity block (K=108) so PSUM holds the final
    x + dy*mask directly; one DVE/ACT copy per 4 rows evacuates the bank
    ([0:108] incl. 20-partition zero lanes -- free-size-bound, same cost).
  - relu+bias evac alternates ScalarE/VectorE per row-pair so neither
    engine is the bottleneck.
  - Structure keeps per-instruction sync-wait fan-in within ISA budgets
    via the NoOp wait-splitting pass (see _split_sync_waits).
"""

import os
import sys

import numpy as np

for _p in ("/opt/trn_rl_repo", os.path.expanduser("~/.axon_site/_ro/trn_rl_repo")):
    if os.path.isdir(os.path.join(_p, "concourse")) and _p not in sys.path:
        sys.path.insert(0, _p)

import concourse.bass as bass
import concourse.mybir as mybir
import concourse.tile as tile
import concourse.tile_sem_assignment as _tsa
from contextlib import ExitStack

# Funnel all DMA completion semaphores onto one HWDGE + one SWDGE lane.
# Same-ring DMAs complete (sem-inc) in FIFO issue order, so a single
# counting lane is sound, and it caps the per-instruction sync-wait
# fan-in (TRN2 ISA allows only 1 wait on a DMA, 2 on a matmul; every
# distinct lane costs a wait slot).
_tsa.NUM_HWDGE_SEMS = 1
_tsa.NUM_SWDGE_GLOBAL_SEMS = 1

C = 12
HID = 96
NCORES = 8
K1 = 109         # 9 shifted x copies (108 partitions) + 1 mask row
KC2 = HID + C    # conv2 contraction: [W2^T; I12] -> 108
BIG_NEG = -1.0e6
FP = mybir.dt.float32
BF = mybir.dt.bfloat16

_IDENT = np.array([[0., 0., 0.], [0., 1., 0.], [0., 0., 0.]], np.float32)
_SOBX = np.array([[-1., 0., 1.], [-2., 0., 2.], [-1., 0., 1.]], np.float32)
_SOBY = _SOBX.T
_LAP = np.array([[1., 2., 1.], [2., -12., 2.], [1., 2., 1.]], np.float32)
FILTERS = np.stack([_IDENT, _SOBX, _SOBY, _LAP])  # [4,3,3]

WALLF = HID + 32  # packed weight-wall free size (128)


def host_weights(w1_w, w1_b, w2_w):
    """Pack all lhsT weight mats into one bf16 [128, 128] wall + the bias.

    wall[p, 0:96]    = conv1 taps: p = (kx*3+ky)*12+c -> w1c[:, c, ky, kx]
    wall[108, 0:96]  = 1.0 (mask-penalty row)
    wall[0:96, 96:108]   = W2^T
    wall[96:108, 96:108] = I12  (residual)
    wall[*, 108:128]     = 0    (pad so conv2 matmuls write full 32-blocks)
    """
    w1r = np.asarray(w1_w, np.float32).reshape(HID, C, 4)
    w1c = np.einsum("ocf,fab->ocab", w1r, FILTERS)  # [96,12,3,3]

    wall = np.zeros((128, WALLF), np.float32)
    for kx in range(3):
        for ky in range(3):
            for c in range(C):
                wall[(kx * 3 + ky) * C + c, 0:HID] = w1c[:, c, ky, kx]
    wall[108, 0:HID] = 1.0                                  # mask-penalty row

    wall[:HID, HID:HID + C] = np.asarray(w2_w, np.float32).T
    wall[HID:KC2, HID:HID + C] = np.eye(C, dtype=np.float32)
    b1 = np.asarray(w1_b, np.float32).reshape(HID, 1).copy()
    return wall, b1


def build_nc(H=512, W=512, R=16, **_ignored):
    """Build the per-core Bass program.

    R: rows per processing chunk (psum pipeline is 2-row pairs / 4-row
    groups inside a chunk).
    """
    PW = W + 2
    RPP = max(1, H // 128)     # rand_u rows per partition in the t image
    PT = H // RPP
    PB = 64                    # prologue rows per pass
    assert H % R == 0 and R % 4 == 0 and R % RPP == 0 and H % PB == 0

    nc = bass.Bass()
    x_d = nc.declare_dram_parameter("x", [C, H, W], FP, isOutput=False)
    u_d = nc.declare_dram_parameter("u", [H, W], FP, isOutput=False)
    wall_d = nc.declare_dram_parameter("wall", [128, WALLF], BF,
                                       isOutput=False)
    b1_d = nc.declare_dram_parameter("b1", [HID, 1], FP, isOutput=False)
    out_d = nc.declare_dram_parameter("out", [C, H, W], FP, isOutput=True)

    AF = mybir.ActivationFunctionType
    AL = mybir.AluOpType

    with tile.TileContext(nc) as tc:
        with ExitStack() as ctx:
            dpool = ctx.enter_context(
                tc.tile_pool(name="dram", bufs=1, space="DRAM"))
            xpad = dpool.tile([C, (H + 2) * PW], BF, tag="xpad")
            xp2 = xpad[:, :].rearrange("c (r w) -> c r w", w=PW)
            xp_t = xpad[:, :].tensor
            xp_base = xpad[:, :].offset
            xflat = dpool.tile([C, H * W], BF, tag="xflat")

            consts = ctx.enter_context(tc.tile_pool(name="consts", bufs=1))
            tpool = ctx.enter_context(tc.tile_pool(name="timg", bufs=1))

            # ---- Prologue B first: weights + mask image, so chunk 0's
            # dependencies (wall, b1, t_dram) clear while the bulkier
            # xpad2 staging below is still streaming.
            wall_sb = consts.tile([128, WALLF], BF, tag="wall")
            nc.sync.dma_start(wall_sb[:], wall_d[:, :])
            b1_sb = consts.tile([HID, 1], FP, tag="b1")
            nc.sync.dma_start(b1_sb[:], b1_d[:, :])

            u_sb = tpool.tile([PT, RPP * W], FP, tag="u")
            nc.sync.dma_start(
                u_sb[:], u_d[:, :].rearrange("(p q) w -> p (q w)", q=RPP))
            # Mask image built PW-strided in SBUF (pad cols zeroed) so the
            # DRAM copy is flat and each chunk's mask load is ONE
            # contiguous descriptor.
            t_sb = tpool.tile([PT, RPP * PW], BF, tag="t")
            nc.vector.memset(t_sb[:], 0.0)
            nc.vector.tensor_scalar(
                t_sb[:, :].rearrange("p (q w) -> p q w", w=PW)[:, :, 0:W],
                u_sb[:, :].rearrange("p (q w) -> p q w", w=W),
                0.5, BIG_NEG, op0=AL.is_lt, op1=AL.mult)
            t_dram = dpool.tile([H, PW], BF, tag="t_dram")
            nc.gpsimd.dma_start(
                t_dram[:, :].rearrange("(p q) w -> p (q w)", q=RPP),
                t_sb[:, :])
            t_flat = t_dram[:, :].rearrange("r w -> (r w)")

            # ---- Prologue A: build xpad2 = circularly padded bf16 x in
            # DRAM, plus an unpadded flat copy xflat for the residual
            # loads.  Staging tiles use a [(c, rowblock), rows*W] layout:
            # every DMA descriptor is a multi-KB contiguous run and all 16
            # SDMA engines participate.
            RB = PB // 8                        # rows per partition block
            with tc.tile_pool(name="prolog", bufs=2) as ppool:
                for p0 in range(0, H, PB):
                    s1 = ppool.tile([C * 8, RB * W], FP, tag="s1")
                    nc.sync.dma_start(
                        s1[:, :],
                        x_d[:, p0:p0 + PB, :].rearrange(
                            "c (b r) w -> c b (r w)", b=8))
                    s2 = ppool.tile([C * 8, RB * PW], BF, tag="s2")
                    s1v = s1[:, :].rearrange("p (r w) -> p r w", w=W)
                    s2v = s2[:, :].rearrange("p (r w) -> p r w", w=PW)
                    nc.vector.tensor_copy(s2v[:, :, 1:W + 1], s1v[:, :, :])
                    nc.vector.tensor_copy(s2v[:, :, 0:1],
                                          s1v[:, :, W - 1:W])
                    nc.vector.tensor_copy(s2v[:, :, W + 1:W + 2],
                                          s1v[:, :, 0:1])
                    # Stores via SWDGE: their waits on the DVE padding must
                    # not stall the SP queue issuing the next pass load.
                    nc.gpsimd.dma_start(
                        xp2[:, p0 + 1:p0 + PB + 1, :].rearrange(
                            "c (b r) w -> c b (r w)", b=8),
                        s2[:, :])
                    nc.gpsimd.dma_start(
                        xflat[:, p0 * W:(p0 + PB) * W].rearrange(
                            "c (b rw) -> c b rw", b=8),
                        s2v[:, :, 1:W + 1])
            # Vertical wrap rows: row 0 <- x row H-1, row H+1 <- x row 0.
            nc.gpsimd.dma_start(xp2[:, 0:1, :], xp2[:, H:H + 1, :])
            nc.gpsimd.dma_start(xp2[:, H + 1:H + 2, :], xp2[:, 1:2, :])

            xpool = ctx.enter_context(tc.tile_pool(name="xbuf", bufs=3))
            hpool = ctx.enter_context(tc.tile_pool(name="h", bufs=2))
            opool = ctx.enter_context(tc.tile_pool(name="ostage", bufs=2))
            ph_pool = ctx.enter_context(
                tc.tile_pool(name="psum_h", bufs=3, space="PSUM"))
            po_pool = ctx.enter_context(
                tc.tile_pool(name="psum_o", bufs=2, space="PSUM"))

            wc1 = [wall_sb[0:K1, 32 * t:32 * t + 32] for t in range(3)]
            wc2 = wall_sb[0:KC2, HID:HID + 32]

            n_chunks = H // R
            # Interior chunks first: chunks 0 and last read the vertical
            # wrap rows written at the very end of the prologue.
            order = list(range(1, n_chunks - 1)) + [0, n_chunks - 1]
            for ci in order:
                r0 = ci * R
                xb = xpool.tile([K1, R * PW], BF, tag="xb")

                # Window loads materialize all 9 (kx,ky) taps: partition
                # p = (kx*3+ky)*12+c holds, at free f = r*PW+w, the value
                # xpad[c, r0+r+ky, w+kx].  One DMA per kx (the AP
                # balancer caps at 3 dims).
                cnt = R * PW - 2
                for kx in range(3):
                    src = bass.AP(
                        xp_t, xp_base + r0 * PW + kx,
                        [[PW, 3], [(H + 2) * PW, C], [1, cnt]])
                    nc.sync.dma_start(out=xb[36 * kx:36 * kx + 36, 0:cnt],
                                      in_=src)

                # Mask rows into partition 108: one contiguous descriptor
                # thanks to the [H, PW] mask layout.
                nc.sync.dma_start(
                    out=xb[K1 - 1:K1, :],
                    in_=t_flat[r0 * PW:(r0 + R) * PW])

                # h chunk; partitions 96:108 hold x rows for the residual
                # (the I12 block of the conv2 weights adds them back).
                # From xflat: one 16 KB contiguous descriptor per channel.
                h = hpool.tile([KC2, R * W], BF, tag="h")
                nc.sync.dma_start(
                    out=h[HID:KC2, :],
                    in_=xflat[:, r0 * W:(r0 + R) * W])

                ost = opool.tile([128, (R // 4) * W], FP, tag="ost")

                NP = R // 2   # row pairs
                NG = R // 4   # conv2 groups

                def conv1_pair(rp):
                    ph = ph_pool.tile([128, 2 * W], FP, tag="ph",
                                      name=f"ph_{ci}_{rp}")
                    for j in range(2):
                        r = rp * 2 + j
                        rhs = xb[0:K1, r * PW:r * PW + W]
                        for t in range(3):
                            nc.tensor.matmul(
                                ph[32 * t:32 * t + 32, j * W:(j + 1) * W],
                                wc1[t], rhs,
                                start=True, stop=True,
                                tile_position=(0, 32 * t))
                    hs = h[0:HID, rp * 2 * W:(rp + 1) * 2 * W]
                    if rp % 2 == 0:
                        nc.scalar.activation(
                            hs, ph[0:HID, :], AF.Relu, bias=b1_sb[:, 0:1])
                    else:
                        nc.vector.tensor_scalar(
                            hs, ph[0:HID, :], b1_sb[:, 0:1], 0.0,
                            op0=AL.add, op1=AL.max)

                def conv2_group(g):
                    # Tile t handles row t*NG + g, so ost block t collects
                    # NG CONSECUTIVE rows -> chunk-end stores are 8 KB
                    # contiguous descriptors.
                    po = po_pool.tile([128, W], FP, tag="po",
                                      name=f"po_{ci}_{g}")
                    for t in range(4):
                        r = t * NG + g
                        nc.tensor.matmul(
                            po[32 * t:32 * t + 32, :],
                            wc2, h[0:KC2, r * W:(r + 1) * W],
                            start=True, stop=True,
                            tile_position=(0, 32 * t))
                    # Evacuate the whole bank (incl. zero lanes) in one
                    # free-size-bound copy; alternate engines.
                    dst = ost[0:KC2, g * W:(g + 1) * W]
                    if g % 2 == 0:
                        nc.vector.tensor_copy(dst, po[0:KC2, :])
                    else:
                        nc.scalar.copy(dst, po[0:KC2, :])

                # conv2 group g needs h row 3*NG+g, i.e. nearly the whole
                # chunk's relu output, so conv2 strictly trails the conv1
                # pairs (issuing it earlier would stall the PE FIFO on a
                # relu whose matmuls are queued behind it); across chunks
                # the pools keep the PE busy.
                for rp in range(NP):
                    conv1_pair(rp)
                for g in range(NG):
                    conv2_group(g)

                # Chunk-end stores: one DMA per column-tile block t; block
                # t holds rows r0+t*NG .. r0+(t+1)*NG.
                for t in range(4):
                    nc.gpsimd.dma_start(
                        out=out_d[:, r0 + t * NG:r0 + (t + 1) * NG, :],
                        in_=ost[32 * t:32 * t + C, :].rearrange(
                            "c (g w) -> c g w", w=W))

    return nc


def _wait_budget(inst):
    return 1


def _split_sync_waits(nc):
    """Move excess per-instruction sem waits onto preceding NoOps.

    The TRN2 ISA caps sync-wait commands per instruction (1 for the DMA
    pseudo-instructions, ~2 elsewhere); walrus refuses to compile above
    the cap. A NoOp on the same engine queue executes its wait in program
    order before the real instruction, so spreading is semantically
    identical.
    """
    import bass_rust

    n = 0
    for fn in nc.m.functions:
        for bb in fn.blocks:
            insts = bb.instructions
            out = []
            for inst in insts:
                si = inst.sync_info
                budget = _wait_budget(inst)
                if si is not None and len(si.on_wait) > budget:
                    waits = list(si.on_wait)
                    excess = waits[:len(waits) - budget]
                    keep = waits[len(waits) - budget:]
                    for w in excess:
                        n += 1
                        nop = mybir.InstNoOp(name=f"wsplit_{n}", ins=[],
                                             outs=[])
                        nop.engine = inst.engine
                        nop.sync_info = bass_rust.SyncInfo(
                            on_wait=[w], on_update=[])
                        out.append(nop)
                    inst.sync_info = bass_rust.SyncInfo(
                        on_wait=keep, on_update=list(si.on_update))
                out.append(inst)
            insts.clear()
            insts.extend(out)
    return n


_NC_CACHE = {}


def _get_nc(**kw):
    kw.pop("f32r", None)
    kw.pop("act_pairs", None)
    key = tuple(sorted(kw.items()))
    if key not in _NC_CACHE:
        nc = build_nc(**kw)
        # Wait-splitting breaks CoreSim's accounting, so it is applied
        # only on the hardware path (here), not inside build_nc.
        _split_sync_waits(nc)
        _NC_CACHE[key] = nc
    return _NC_CACHE[key]


def run(x, w1_w, w1_b, w2_w, rand_u, trace=False, **build_kw):
    """Shard over batch, run on 8 cores, gather. Returns (out, results)."""
    from concourse.bass_utils import run_bass_kernel_spmd

    import ml_dtypes

    x = np.ascontiguousarray(np.asarray(x, np.float32))
    rand_u = np.ascontiguousarray(np.asarray(rand_u, np.float32))
    b, c, hh, ww = x.shape
    assert b == NCORES and c == C
    wall, b1 = host_weights(w1_w, w1_b, w2_w)
    wall = wall.astype(ml_dtypes.bfloat16)

    nc = _get_nc(H=hh, W=ww, **build_kw)
    in_maps = [
        {
            "x": x[i],
            "u": rand_u[i, 0],
            "wall": wall,
            "b1": b1,
        }
        for i in range(NCORES)
    ]
    res = run_bass_kernel_spmd(nc, in_maps, list(range(NCORES)), trace=trace)
    out = np.stack([res.results[i]["out"] for i in range(NCORES)])
    return out.astype(np.float32), res


def kernel(x, w1_w, w1_b, w2_w, rand_u):
    out, _ = run(x, w1_w, w1_b, w2_w, rand_u)
    return out
